# revision 1
# baseline (speedup 1.0000x reference)
import os
import sys
from contextlib import ExitStack

import numpy as np
from math import gcd as ngcd

sys.path.insert(0, "/opt/trn_rl_repo")

# Problem constants (hardcoded per harness contract)
N, E, D, H, DK, FF = 50000, 1600000, 128, 8, 16, 512
P = 8
NS = N // P          # 6250 nodes per core (dst shard)
HALF = N // 2        # 25000 (src half for int16 gather indices)
CH = 2048            # edge slots per chunk
EPS = 1e-5

# feature permutation: f' = j*8 + h  (head index innermost) so that per-head
# scalars broadcast along a stride-1 inner dim of length 8.
_FP = np.arange(D)
SRC_COL = (_FP % 8) * 16 + _FP // 8   # source feature for permuted position f'


def _wrap16(a):
    """int16 index array -> [128, L/16]: wrapped in 16 partitions, replicated
    across the 8 gpsimd cores (HW idx layout for dma_gather/scatter_add)."""
    a = np.asarray(a, np.int16)
    assert a.size % 16 == 0
    return np.ascontiguousarray(np.tile(a.reshape(-1, 16).T, (8, 1)))


def _prep_core(src, dst, c, n_chunks):
    """Build per-core slot->edge assignment, conflict-free within each chunk.

    Returns (kv_idx, qd_idx, sc_idx) int16 arrays of length 2*n_chunks*CH
    (pass A chunks then pass B chunks)."""
    sel = (dst // NS) == c
    es, ed = src[sel], dst[sel] - c * NS
    kv_idx = np.zeros(2 * n_chunks * CH, np.int16)
    qd_idx = np.zeros(2 * n_chunks * CH, np.int16)
    sc_idx = np.full(2 * n_chunks * CH, NS, np.int16)  # junk row by default
    for p in range(2):
        psel = (es >= HALF) == bool(p)
        s_, d_ = es[psel], ed[psel]
        # order edges by dst so we can round-robin each node's edges over
        # chunks with stride 5 (no same-chunk or adjacent-chunk duplicates)
        order = np.argsort(d_, kind="stable")
        s_, d_ = s_[order], d_[order]
        # rank of each edge within its dst node
        first = np.r_[True, d_[1:] != d_[:-1]]
        idx_of_first = np.maximum.accumulate(np.where(first, np.arange(d_.size), 0))
        rank = np.arange(d_.size) - idx_of_first
        assert rank.max() < n_chunks, f"max degree {rank.max()+1} > n_chunks"
        chunk = ((rank * 5 + d_ * 7) % n_chunks).astype(np.int64)
        # rebalance overflowing chunks: move excess edges to underfull chunks
        # that don't already contain the same dst (scatter conflict-freedom)
        fill = np.bincount(chunk, minlength=n_chunks)
        if fill.max() > CH:
            used = [set() for _ in range(n_chunks)]
            for i in range(d_.size):
                used[chunk[i]].add(int(d_[i]))
            order2 = np.argsort(fill[chunk], kind="stable")[::-1]
            for i in order2:
                c0 = chunk[i]
                if fill[c0] <= CH:
                    continue
                dv = int(d_[i])
                for c1 in np.argsort(fill):
                    if fill[c1] < CH and dv not in used[c1]:
                        used[c0].discard(dv)
                        used[c1].add(dv)
                        fill[c0] -= 1
                        fill[c1] += 1
                        chunk[i] = c1
                        break
            assert fill.max() <= CH, "rebalance failed"
        pos = np.zeros(d_.size, np.int64)
        byc = np.argsort(chunk, kind="stable")
        cc = chunk[byc]
        starts = np.r_[True, cc[1:] != cc[:-1]]
        sidx = np.maximum.accumulate(np.where(starts, np.arange(cc.size), 0))
        within = np.arange(cc.size) - sidx
        pos[byc] = within
        assert pos.max() < CH, f"chunk overflow {pos.max()+1} > {CH}"
        slot = (p * n_chunks + chunk) * CH + pos
        kv_idx[slot] = (s_ - p * HALF).astype(np.int16)
        qd_idx[slot] = d_.astype(np.int16)
        sc_idx[slot] = d_.astype(np.int16)
    return kv_idx, qd_idx, sc_idx


def kernel(**inputs):
    import concourse.bass as bass
    import concourse.tile as tile
    from concourse import bacc
    from concourse import mybir
    from concourse.bass_utils import run_bass_kernel_spmd
    from concourse.library_config import mlp as mlp_lib
    import ml_dtypes

    bf16 = ml_dtypes.bfloat16
    f32 = mybir.dt.float32
    bf = mybir.dt.bfloat16
    i16 = mybir.dt.int16
    AF = mybir.ActivationFunctionType

    h = np.ascontiguousarray(np.asarray(inputs["h"], np.float32))
    src = np.asarray(inputs["src"], np.int32)
    dst = np.asarray(inputs["dst"], np.int32)
    Wq = np.asarray(inputs["Wq"], np.float32)[:, SRC_COL]
    Wk = np.asarray(inputs["Wk"], np.float32)[:, SRC_COL]
    Wv = np.asarray(inputs["Wv"], np.float32)[:, SRC_COL]
    Wo = np.asarray(inputs["Wo"], np.float32)[SRC_COL, :]
    ln1_g = np.asarray(inputs["ln1_g"], np.float32)
    ln1_b = np.asarray(inputs["ln1_b"], np.float32)
    ln2_g = np.asarray(inputs["ln2_g"], np.float32)
    ln2_b = np.asarray(inputs["ln2_b"], np.float32)
    W1 = np.asarray(inputs["W1"], np.float32)
    b1 = np.asarray(inputs["b1"], np.float32)
    W2 = np.asarray(inputs["W2"], np.float32)
    b2 = np.asarray(inputs["b2"], np.float32)

    # host-side sharding / index prep ------------------------------------
    deg = np.bincount(dst, minlength=N)
    max_half_deg = 0
    n_chunks = 0
    # per-core counts to size the chunk grid uniformly across cores
    for c in range(P):
        sel = (dst // NS) == c
        for p in range(2):
            cnt = int(((src[sel] >= HALF) == bool(p)).sum())
            n_chunks = max(n_chunks, (cnt + CH - 1) // CH)
    # also must exceed max per-(core,pass) node degree for conflict-freedom
    for p in range(2):
        dd = np.bincount(dst[(src >= HALF) == bool(p)], minlength=N)
        max_half_deg = max(max_half_deg, int(dd.max()))
    n_chunks = max(n_chunks + 2, max_half_deg + 2)
    while ngcd(n_chunks, 5) != 1 or ngcd(n_chunks, 7) != 1:
        n_chunks += 1
    L = 2 * n_chunks * CH

    skip_ln1 = not (np.any(ln1_g != 1.0) or np.any(ln1_b != 0.0))
    skip_ln2 = not (np.any(ln2_g != 1.0) or np.any(ln2_b != 0.0))

    # device program ------------------------------------------------------
    nc = bacc.Bacc(None)
    h_full = nc.declare_dram_parameter("h_full", [N, D], f32, isOutput=False)
    h_shard = nc.declare_dram_parameter("h_shard", [NS, D], f32, isOutput=False)
    # [Wk|Wv|Wq] bf16 pre-permuted
    wkvq = nc.declare_dram_parameter("wkvq", [D, 3 * D], bf, isOutput=False)
    wo_p = nc.declare_dram_parameter("wo_p", [D, D], bf, isOutput=False)
    w1_p = nc.declare_dram_parameter("w1_p", [D, FF], bf, isOutput=False)
    w2_p = nc.declare_dram_parameter("w2_p", [FF, D], bf, isOutput=False)
    ident = nc.declare_dram_parameter("ident", [128, 128], bf, isOutput=False)
    ident32 = nc.declare_dram_parameter("ident32", [128, 128], f32, isOutput=False)
    ln_rep = nc.declare_dram_parameter("ln_rep", [128, 5 * D], f32, isOutput=False)
    bias_ff = nc.declare_dram_parameter("bias_ff", [FF, 2], f32, isOutput=False)
    kv_idx = nc.declare_dram_parameter("kv_idx", [128, L // 16], i16, isOutput=False)
    qd_idx = nc.declare_dram_parameter("qd_idx", [128, L // 16], i16, isOutput=False)
    sc_idx = nc.declare_dram_parameter("sc_idx", [128, L // 16], i16, isOutput=False)
    out = nc.declare_dram_parameter("out", [NS, D], f32, isOutput=True)

    kv_table = nc.dram_tensor("kv_table", [N, 2 * D], bf)
    q_table = nc.dram_tensor("q_table", [NS, D], bf)
    acc_num = nc.dram_tensor("acc_num", [NS + 1, D], bf)
    acc_den = nc.dram_tensor("acc_den", [NS + 1, 64], f32)

    with tile.TileContext(nc) as tc, ExitStack() as ctx:
        nc.gpsimd.load_library(mlp_lib)
        cpool = ctx.enter_context(tc.tile_pool(name="consts", bufs=1))
        wkvq_sb = cpool.tile([128, 3 * D], bf)
        nc.sync.dma_start(wkvq_sb[:], wkvq[:, :])
        wo_sb = cpool.tile([128, D], bf)
        nc.sync.dma_start(wo_sb[:], wo_p[:, :])
        w1_sb = cpool.tile([128, FF], bf)
        nc.sync.dma_start(w1_sb[:], w1_p[:, :])
        w2_sb = cpool.tile([128, 4, D], bf)  # [128 ff-inner, chunk, D]
        for k in range(4):
            nc.sync.dma_start(w2_sb[:, k, :], w2_p[k * 128:(k + 1) * 128, :])
        id_sb = cpool.tile([128, 128], bf)
        nc.sync.dma_start(id_sb[:], ident[:, :])
        id32_sb = cpool.tile([128, 128], f32)
        nc.sync.dma_start(id32_sb[:], ident32[:, :])
        ln_sb = cpool.tile([128, 5 * D], f32)
        nc.sync.dma_start(ln_sb[:], ln_rep[:, :])
        bff_sb = cpool.tile([128, 4, 2], f32)
        for k in range(4):
            nc.sync.dma_start(bff_sb[:, k, :], bias_ff[k * 128:(k + 1) * 128, :])
        zero_sb = cpool.tile([128, 128], f32)
        nc.vector.memset(zero_sb[:], 0)
        zerob_sb = cpool.tile([128, 128], bf)
        nc.vector.memset(zerob_sb[:], 0)
        eps_sb = cpool.tile([128, 1], f32)
        nc.vector.memset(eps_sb[:], EPS)

        # ---- phase 0: zero accumulators --------------------------------
        for t in range((NS + 1 + 127) // 128):
            r0 = t * 128
            nr = min(128, NS + 1 - r0)
            nc.sync.dma_start(acc_num[r0:r0 + nr, :], zerob_sb[0:nr, :])
            nc.sync.dma_start(acc_den[r0:r0 + nr, :], zero_sb[0:nr, 0:64])

        # ---- phase 1: build kv table (all N nodes) and q table (shard) --
        def proj_tiles(src_dram, n_rows, rhs_ap, dst_dram, dst_cols, label):
          with tc.tile_pool(name=f"p1_{label}", bufs=3) as pool, \
               tc.tile_pool(name=f"p1ps_{label}", bufs=2, space="PSUM") as pspool:
            n_t = (n_rows + 127) // 128
            for t in range(n_t):
                r0 = t * 128
                nr = min(128, n_rows - r0)
                hin = pool.tile([128, D], f32, tag="hin")
                nc.sync.dma_start(hin[0:nr, :], src_dram[r0:r0 + nr, :])
                hps = pspool.tile([128, 128], f32, tag="hps")
                nc.tensor.transpose(hps[:, 0:nr], hin[0:nr, :], id32_sb[0:nr, 0:nr])
                hT = pool.tile([128, 128], bf, tag="hT")
                nc.vector.tensor_copy(hT[:, 0:nr], hps[:, 0:nr])
                ops = pspool.tile([128, dst_cols], f32, tag="ops")
                nc.tensor.matmul(ops[0:nr, :], hT[:, 0:nr], rhs_ap)
                ot = pool.tile([128, dst_cols], bf, tag="ot")
                nc.scalar.copy(ot[0:nr, :], ops[0:nr, :])
                nc.sync.dma_start(dst_dram[r0:r0 + nr, :], ot[0:nr, :])

        proj_tiles(h_full, N, wkvq_sb[:, 0:2 * D], kv_table, 2 * D, "kv")
        proj_tiles(h_shard, NS, wkvq_sb[:, 2 * D:3 * D], q_table, D, "q")

        # ---- phase 2: edge chunks --------------------------------------
        DBG = int(os.environ.get('KDBG', '0'))
        G = CH // 128
        epool = ctx.enter_context(tc.tile_pool(name="p2", bufs=4))
        ipool = ctx.enter_context(tc.tile_pool(name="p2i", bufs=4))
        for ci in range([2 * n_chunks, 0, 2, 2, 2 * n_chunks, 2 * n_chunks][DBG]):
            p = ci // n_chunks
            i0 = ci * (CH // 16)
            kvi = ipool.tile([128, CH // 16], i16, tag="kvi")
            nc.sync.dma_start(kvi[:], kv_idx[:, i0:i0 + CH // 16])
            qdi = ipool.tile([128, CH // 16], i16, tag="qdi")
            nc.sync.dma_start(qdi[:], qd_idx[:, i0:i0 + CH // 16])
            sci = ipool.tile([128, CH // 16], i16, tag="sci")
            nc.sync.dma_start(sci[:], sc_idx[:, i0:i0 + CH // 16])

            kvt = epool.tile([128, G, 2 * D], bf, tag="kvt")
            src_ap = kv_table[p * HALF:, :] if p else kv_table[0:HALF + NS, :]
            nc.gpsimd.dma_gather(kvt[:], src_ap, kvi[:], CH, CH, 2 * D,
                                 single_packet=False)
            qdt = epool.tile([128, G, D], bf, tag="qdt")
            nc.gpsimd.dma_gather(qdt[:], q_table[:, :], qdi[:], CH, CH, D,
                                 single_packet=False)

            pr = epool.tile([128, G, D], bf, tag="pr")
            nc.vector.tensor_mul(pr[:], qdt[:], kvt[:, :, 0:D])
            pv = pr[:].rearrange("p g (j h) -> p g j h", j=DK, h=H)
            nc.vector.tensor_add(pv[:, :, 0:8, :], pv[:, :, 0:8, :], pv[:, :, 8:16, :])
            nc.vector.tensor_add(pv[:, :, 0:4, :], pv[:, :, 0:4, :], pv[:, :, 4:8, :])
            nc.vector.tensor_add(pv[:, :, 0:2, :], pv[:, :, 0:2, :], pv[:, :, 2:4, :])
            e32 = epool.tile([128, G, H], f32, tag="e32")
            nc.vector.tensor_add(e32[:].unsqueeze(2), pv[:, :, 0:1, :], pv[:, :, 1:2, :])
            ex32 = epool.tile([128, G, H], f32, tag="ex32")
            nc.scalar.activation(ex32[:], e32[:], AF.Exp, scale=0.25,
                                 bias=zero_sb[:, 0:1])
            exb = epool.tile([128, G, H], bf, tag="exb")
            nc.scalar.copy(exb[:], ex32[:])
            y32 = epool.tile([128, G, D], bf, tag="y32")
            nc.vector.tensor_mul(
                y32[:].rearrange("p g (j h) -> p g j h", j=DK, h=H),
                kvt[:, :, D:2 * D].rearrange("p g (j h) -> p g j h", j=DK, h=H),
                exb[:].unsqueeze(2).broadcast_to([128, G, DK, H]))
            if DBG not in (3, 4):
                nc.gpsimd.dma_scatter_add(acc_num[:, :], y32[:], sci[:], CH, CH, D,
                                     single_packet=False)
            if DBG not in (2, 4):
                nc.gpsimd.dma_scatter_add(
                    acc_den[:, 0:8], ex32[:], sci[:], CH, CH, 8, elem_step=64,
                    single_packet=False)

        # ---- phase 3: normalize + Wo + LN1 + FFN + LN2 ------------------
        tpool = ctx.enter_context(tc.tile_pool(name="p3", bufs=3))
        tps = ctx.enter_context(tc.tile_pool(name="p3ps", bufs=3, space="PSUM"))
        n_t = (NS + 127) // 128
        for t in range(n_t):
            r0 = t * 128
            nr = min(128, NS - r0)
            numt = tpool.tile([128, D], bf, tag="numt")
            nc.sync.dma_start(numt[0:nr, :], acc_num[r0:r0 + nr, :])
            dent = tpool.tile([128, 64], f32, tag="dent")
            nc.sync.dma_start(dent[0:nr, :], acc_den[r0:r0 + nr, :])
            dmx = tpool.tile([128, H], f32, tag="dmx")
            nc.vector.tensor_scalar_max(dmx[0:nr, :], dent[0:nr, 0:H], 1e-30)
            rden = tpool.tile([128, H], f32, tag="rden")
            nc.vector.reciprocal(rden[0:nr, :], dmx[0:nr, :])
            a16 = tpool.tile([128, D], bf, tag="a16")
            nc.vector.tensor_mul(
                a16[0:nr, :].rearrange("p (j h) -> p j h", j=DK, h=H),
                numt[0:nr, :].rearrange("p (j h) -> p j h", j=DK, h=H),
                rden[0:nr, :].unsqueeze(1).broadcast_to([nr, DK, H]))
            aps = tps.tile([128, 128], bf, tag="tbf")
            nc.tensor.transpose(aps[:, 0:nr], a16[0:nr, :], id_sb[0:nr, 0:nr])
            aT = tpool.tile([128, 128], bf, tag="aT")
            nc.vector.tensor_copy(aT[:, 0:nr], aps[:, 0:nr])
            ops_ = tps.tile([128, D], f32, tag="tf32")
            nc.tensor.matmul(ops_[0:nr, :], aT[:, 0:nr], wo_sb[:])
            hsb = tpool.tile([128, D], f32, tag="hsb")
            nc.sync.dma_start(hsb[0:nr, :], h_shard[r0:r0 + nr, :])
            x = tpool.tile([128, D], f32, tag="x")
            nc.vector.tensor_add(x[0:nr, :], ops_[0:nr, :], hsb[0:nr, :])

            def layernorm(xin, g_off, skip, tag):
                mu = tpool.tile([128, 1], f32, tag=f"mu{tag}")
                nc.vector.reduce_sum(mu[0:nr, :], xin[0:nr, :],
                                     axis=mybir.AxisListType.X)
                nc.vector.tensor_scalar_mul(mu[0:nr, :], mu[0:nr, :], 1.0 / D)
                xc = tpool.tile([128, D], f32, tag=f"xc{tag}")
                nc.vector.tensor_scalar_sub(xc[0:nr, :], xin[0:nr, :], mu[0:nr, :])
                sq = tpool.tile([128, D], f32, tag=f"sq{tag}")
                nc.vector.tensor_mul(sq[0:nr, :], xc[0:nr, :], xc[0:nr, :])
                var = tpool.tile([128, 1], f32, tag=f"var{tag}")
                nc.vector.reduce_sum(var[0:nr, :], sq[0:nr, :],
                                     axis=mybir.AxisListType.X)
                std = tpool.tile([128, 1], f32, tag=f"std{tag}")
                nc.scalar.activation(std[0:nr, :], var[0:nr, :], AF.Sqrt,
                                     scale=1.0 / D, bias=eps_sb[0:nr, :])
                rstd = tpool.tile([128, 1], f32, tag=f"rstd{tag}")
                nc.vector.reciprocal(rstd[0:nr, :], std[0:nr, :])
                y = tpool.tile([128, D], f32, tag=f"y{tag}")
                nc.vector.tensor_scalar_mul(y[0:nr, :], xc[0:nr, :], rstd[0:nr, :])
                if not skip:
                    nc.vector.tensor_mul(y[0:nr, :], y[0:nr, :],
                                         ln_sb[0:nr, g_off:g_off + D])
                    nc.vector.tensor_add(y[0:nr, :], y[0:nr, :],
                                         ln_sb[0:nr, g_off + D:g_off + 2 * D])
                return y

            h1 = layernorm(x, 0, skip_ln1, "1")
            h1b = tpool.tile([128, D], bf, tag="h1b")
            nc.vector.tensor_copy(h1b[0:nr, :], h1[0:nr, :])
            h1ps = tps.tile([128, 128], bf, tag="tbf")
            nc.tensor.transpose(h1ps[:, 0:nr], h1b[0:nr, :], id_sb[0:nr, 0:nr])
            h1T = tpool.tile([128, 128], bf, tag="h1T")
            nc.vector.tensor_copy(h1T[:, 0:nr], h1ps[:, 0:nr])
            f2ps = tps.tile([128, 128], f32, tag="tf32")
            for k in range(4):
                fps = tps.tile([128, 128], f32, tag="tf32")
                nc.tensor.matmul(fps[:, 0:nr], w1_sb[:, k * 128:(k + 1) * 128],
                                 h1T[:, 0:nr])
                rl = tpool.tile([128, 128], bf, tag="rl")
                nc.scalar.activation(rl[:, 0:nr], fps[:, 0:nr], AF.Relu,
                                     bias=bff_sb[:, k, 0:1])
                nc.tensor.matmul(f2ps[:, 0:nr], w2_sb[:, k, :], rl[:, 0:nr],
                                 start=(k == 0), stop=(k == 3))
            f2b = tpool.tile([128, 128], bf, tag="f2b")
            nc.scalar.activation(f2b[:, 0:nr], f2ps[:, 0:nr], AF.Copy)
            fsl = tps.tile([128, 128], bf, tag="tbf")
            nc.tensor.transpose(fsl[0:nr, :], f2b[:, 0:nr], id_sb[:, :])
            x2 = tpool.tile([128, D], f32, tag="x2")
            nc.vector.tensor_add(x2[0:nr, :], fsl[0:nr, :], h1[0:nr, :])
            if np.any(b2 != 0.0):
                nc.vector.tensor_add(x2[0:nr, :], x2[0:nr, :],
                                     ln_sb[0:nr, 4 * D:5 * D])
            h2 = layernorm(x2, 2 * D, skip_ln2, "2")
            nc.sync.dma_start(out[r0:r0 + nr, :], h2[0:nr, :])

    # inputs per core -----------------------------------------------------
    wkvq_np = np.concatenate([Wk, Wv, Wq], axis=1).astype(bf16)
    ln_rep2 = np.zeros((128, 5 * D), np.float32)
    ln_rep2[:, 0:D] = ln1_g
    ln_rep2[:, D:2 * D] = ln1_b
    ln_rep2[:, 2 * D:3 * D] = ln2_g
    ln_rep2[:, 3 * D:4 * D] = ln2_b
    ln_rep2[:, 4 * D:5 * D] = b2
    bias_ff_np = np.zeros((FF, 2), np.float32)
    bias_ff_np[:, 0] = b1

    in_maps = []
    for c in range(P):
        kvw, qdw, scw = _prep_core(src, dst, c, n_chunks)
        in_maps.append({
            "h_full": h,
            "h_shard": np.ascontiguousarray(h[c * NS:(c + 1) * NS]),
            "wkvq": wkvq_np,
            "wo_p": Wo.astype(bf16),
            "w1_p": W1.astype(bf16),
            "w2_p": W2.astype(bf16),
            "ident": np.eye(128, dtype=bf16),
            "ident32": np.eye(128, dtype=np.float32),
            "ln_rep": ln_rep2,
            "bias_ff": bias_ff_np,
            "kv_idx": _wrap16(kvw),
            "qd_idx": _wrap16(qdw),
            "sc_idx": _wrap16(scw),
        })

    nc.finalize()
    kernel.last_nc = nc
    res = run_bass_kernel_spmd(nc, in_maps, core_ids=list(range(P)),
                               trace=bool(int(os.environ.get("BASS_TRACE", "0"))))
    kernel.last_results = res
    return np.concatenate([res.results[c]["out"] for c in range(P)], axis=0)



# revision 53
# speedup vs baseline: 3.5837x; 3.5837x over previous
import os
import sys
from contextlib import ExitStack

import numpy as np

sys.path.insert(0, "/opt/trn_rl_repo")

# Problem constants (hardcoded per harness contract)
N, E, D, H, DK, FF = 50000, 1600000, 128, 8, 16, 512
P = 8
NS = N // P            # 6250 nodes per core (dst shard)
GN = 128               # nodes per group (one SBUF partition each)
NG = (NS + GN - 1) // GN   # 49 groups
NSP = NG * GN          # 6272 padded node rows per core
EPS = 1e-5
WCAP = 40              # max gather blocks processed per subtile

# kv table rows: row 0 = zeros, rows 1..N = node r-1, row N+1 = zeros.
# window A = rows [0, 32768)        -> src s uses idx s+1      (s <= 32766)
# window B = rows [17234, 50002)    -> src s uses idx s-17233  (s >= 17233)
ROWS = N + 2
AWIN = 32768
BBASE = 17234
SLO = BBASE - 1        # s <  SLO: strictly window A
SHI = AWIN - 1         # s >= SHI: strictly window B
BPAD = N + 1 - BBASE   # 32767, idx of the zero row in window B

# v feature permutation: f' = jf*8 + h  (head index innermost, stride 1)
_FP = np.arange(D)
SRC_COL = (_FP % 8) * 16 + _FP // 8   # source feature for permuted position f'


def _wrap16(a):
    """int16 index array -> [128, L/16]: wrapped in 16 partitions, replicated
    across the 8 gpsimd cores (HW idx layout for dma_gather)."""
    a = np.asarray(a, np.int16)
    assert a.size % 16 == 0
    return np.ascontiguousarray(np.tile(a.reshape(-1, 16).T, (8, 1)))


def _rank_within(keys):
    """Rank of each element within its key group."""
    order = np.argsort(keys, kind="stable")
    ks = keys[order]
    first = np.r_[True, ks[1:] != ks[:-1]]
    idx_of_first = np.maximum.accumulate(np.where(first, np.arange(ks.size), 0))
    rank = np.arange(ks.size) - idx_of_first
    out = np.empty(keys.size, np.int64)
    out[order] = rank
    return out


def _split(n, k):
    q, r = divmod(n, k)
    return [q + (i < r) for i in range(k)]


def _prep_core(src, dst, c):
    """Per-core prep: balanced A/B window split + grouping by max(dA, dB)."""
    sel = (dst // NS) == c
    es = src[sel].astype(np.int64)
    ed = (dst[sel] - c * NS).astype(np.int64)

    isA = es < SLO
    isB = es >= SHI
    isF = ~(isA | isB)
    na = np.bincount(ed[isA], minlength=NS)
    nb = np.bincount(ed[isB], minlength=NS)
    nf = np.bincount(ed[isF], minlength=NS)
    d = na + nb + nf
    # flex edges sent to A, to balance dA vs dB
    x = np.clip((nb + nf - na + 1) // 2, 0, nf)
    frank = np.zeros(es.size, np.int64)
    frank[isF] = _rank_within(ed[isF])
    toA = isA | (isF & (frank < x[ed]))
    dA = np.bincount(ed[toA], minlength=NS)
    dB = d - dA

    # sort (desc, stable) by per-pass max degree (what actually sets W)
    order = np.argsort(-np.maximum(dA, dB), kind="stable")
    sortpos = np.empty(NS, np.int64)
    sortpos[order] = np.arange(NS)

    dA_s = np.zeros(NSP, np.int64)
    dB_s = np.zeros(NSP, np.int64)
    dA_s[sortpos] = dA
    dB_s[sortpos] = dB
    dAmax = dA_s.reshape(NG, GN).max(axis=1)
    dBmax = dB_s.reshape(NG, GN).max(axis=1)
    return dict(es=es, ed=ed, toA=toA, dA_s=dA_s, dB_s=dB_s,
                order=order, sortpos=sortpos, dAmax=dAmax, dBmax=dBmax)


def _make_subs(WA, WB):
    """Per-group subtile splits (wa_i, wb_i) with wa_i+wb_i <= WCAP, plus
    slot-stream bases."""
    SUBS, subbase = [], []
    off = 0
    for g in range(NG):
        n_sub = 1
        while True:
            was = _split(int(WA[g]), n_sub)
            wbs = _split(int(WB[g]), n_sub)
            if max(a + b for a, b in zip(was, wbs)) <= WCAP:
                break
            n_sub += 1
        SUBS.append(list(zip(was, wbs)))
        bl = []
        for wa, wb in SUBS[g]:
            bl.append((off, off + wa * GN))
            off += (wa + wb) * GN
        subbase.append(bl)
    return SUBS, subbase, off


def _core_slots(prep, WA, WB, SUBS, subbase, TOT):
    """Build the full int16 gather index stream for one core."""
    es, ed, toA = prep["es"], prep["ed"], prep["toA"]
    sortpos = prep["sortpos"]
    idx = np.empty(TOT, np.int16)
    # pad fills, and rank->slot-base LUTs per group
    lutA = np.zeros((NG, int(WA.max())), np.int64)
    lutB = np.zeros((NG, int(WB.max())), np.int64)
    for g in range(NG):
        ra = rb = 0
        for (wa, wb), (bA, bB) in zip(SUBS[g], subbase[g]):
            idx[bA:bA + wa * GN] = 0
            idx[bB:bB + wb * GN] = BPAD
            for j in range(wa):
                lutA[g, ra + j] = bA + j * GN
            for j in range(wb):
                lutB[g, rb + j] = bB + j * GN
            ra += wa
            rb += wb
    for sel, lut, val in (
        (toA, lutA, es + 1),
        (~toA, lutB, es - (BBASE - 1)),
    ):
        pos = sortpos[ed[sel]]
        g = pos // GN
        p = pos % GN
        r = _rank_within(pos)
        slot = lut[g, r] + p
        idx[slot] = val[sel].astype(np.int16)
    return idx


def kernel(**inputs):
    import concourse.bass as bass
    import concourse.tile as tile
    from concourse import bacc
    from concourse import mybir
    from concourse.bass_utils import run_bass_kernel_spmd
    from concourse.library_config import mlp as mlp_lib
    from concourse.alu_op_type import AluOpType
    import ml_dtypes

    bf16 = ml_dtypes.bfloat16
    f32 = mybir.dt.float32
    bf = mybir.dt.bfloat16
    i16 = mybir.dt.int16
    AF = mybir.ActivationFunctionType
    AX = mybir.AxisListType

    h = np.ascontiguousarray(np.asarray(inputs["h"], np.float32))
    src = np.asarray(inputs["src"], np.int32)
    dst = np.asarray(inputs["dst"], np.int32)
    Wq = np.asarray(inputs["Wq"], np.float32)
    Wk = np.asarray(inputs["Wk"], np.float32)
    Wv = np.asarray(inputs["Wv"], np.float32)[:, SRC_COL]
    Wo = np.asarray(inputs["Wo"], np.float32)[SRC_COL, :]
    ln1_g = np.asarray(inputs["ln1_g"], np.float32)
    ln1_b = np.asarray(inputs["ln1_b"], np.float32)
    ln2_g = np.asarray(inputs["ln2_g"], np.float32)
    ln2_b = np.asarray(inputs["ln2_b"], np.float32)
    W1 = np.asarray(inputs["W1"], np.float32)
    b1 = np.asarray(inputs["b1"], np.float32)
    W2 = np.asarray(inputs["W2"], np.float32)
    b2 = np.asarray(inputs["b2"], np.float32)

    # ---- host-side sharding / index prep --------------------------------
    preps = [_prep_core(src, dst, c) for c in range(P)]
    WA = np.maximum(np.max([p["dAmax"] for p in preps], axis=0), 1)
    WB = np.maximum(np.max([p["dBmax"] for p in preps], axis=0), 1)
    SUBS, subbase, TOT = _make_subs(WA, WB)
    TOTC = TOT // 16

    skip_ln1 = not (np.any(ln1_g != 1.0) or np.any(ln1_b != 0.0))
    skip_ln2 = not (np.any(ln2_g != 1.0) or np.any(ln2_b != 0.0))
    add_b1 = bool(np.any(b1 != 0.0))
    add_b2 = bool(np.any(b2 != 0.0))

    # ---- device program --------------------------------------------------
    nc = bacc.Bacc(None)
    hbT = nc.declare_dram_parameter("hbT", [128, ROWS], bf, isOutput=False)
    h_perm = nc.declare_dram_parameter("h_perm", [NSP, D], f32, isOutput=False)
    hT_perm = nc.declare_dram_parameter("hT_perm", [128, NSP], bf, isOutput=False)
    wkv_p = nc.declare_dram_parameter("wkv_p", [D, 2 * D], bf, isOutput=False)
    wq_p = nc.declare_dram_parameter("wq_p", [D, D], bf, isOutput=False)
    wo_p = nc.declare_dram_parameter("wo_p", [D, D], bf, isOutput=False)
    w1_p = nc.declare_dram_parameter("w1_p", [D, FF], bf, isOutput=False)
    w2_p = nc.declare_dram_parameter("w2_p", [FF, D], bf, isOutput=False)
    ident = nc.declare_dram_parameter("ident", [128, 128], bf, isOutput=False)
    ident32 = nc.declare_dram_parameter("ident32", [128, 128], f32, isOutput=False)
    ln_rep = nc.declare_dram_parameter("ln_rep", [128, 5 * D], f32, isOutput=False)
    bias_ff = nc.declare_dram_parameter("bias_ff", [FF, 2], f32, isOutput=False)
    padc = nc.declare_dram_parameter("padc", [128, NG], f32, isOutput=False)
    kv_idx = nc.declare_dram_parameter("kv_idx", [128, TOTC], i16, isOutput=False)
    out = nc.declare_dram_parameter("out", [NSP, D], f32, isOutput=True)

    kv_table = nc.dram_tensor("kv_table", [ROWS, 2 * D], bf)

    with tile.TileContext(nc) as tc, ExitStack() as ctx:
        nc.gpsimd.load_library(mlp_lib)
        cpool = ctx.enter_context(tc.tile_pool(name="consts", bufs=1))
        wkv_sb = cpool.tile([128, 2 * D], bf)
        nc.sync.dma_start(wkv_sb[:], wkv_p[:, :])
        wq_sb = cpool.tile([128, D], bf)
        nc.sync.dma_start(wq_sb[:], wq_p[:, :])
        wo_sb = cpool.tile([128, D], bf)
        nc.sync.dma_start(wo_sb[:], wo_p[:, :])
        w1_sb = cpool.tile([128, FF], bf)
        nc.sync.dma_start(w1_sb[:], w1_p[:, :])
        w2_sb = cpool.tile([128, 4, D], bf)
        for k in range(4):
            nc.sync.dma_start(w2_sb[:, k, :], w2_p[k * 128:(k + 1) * 128, :])
        id_sb = cpool.tile([128, 128], bf)
        nc.sync.dma_start(id_sb[:], ident[:, :])
        id32_sb = cpool.tile([128, 128], f32)
        nc.sync.dma_start(id32_sb[:], ident32[:, :])
        ln_sb = cpool.tile([128, 5 * D], f32)
        nc.sync.dma_start(ln_sb[:], ln_rep[:, :])
        bff_sb = cpool.tile([128, 4, 2], f32)
        for k in range(4):
            nc.sync.dma_start(bff_sb[:, k, :], bias_ff[k * 128:(k + 1) * 128, :])
        padc_sb = cpool.tile([128, NG], f32)
        nc.sync.dma_start(padc_sb[:], padc[:, :])
        eps_sb = cpool.tile([128, 1], f32)
        nc.vector.memset(eps_sb[:], EPS)

        # ---- phase 1: build kv table [k_nat | v_perm] for all table rows -
        KPH = int(os.environ.get("KPH", "0"))  # 1: phase1 only, 2: phase2 only
        CH1 = 16                      # 128-row tiles per chunk
        RCH = CH1 * 128              # 1024 rows per chunk
        nch = ROWS // RCH
        rem = ROWS - nch * RCH
        with tc.tile_pool(name="p1", bufs=3) as pool1, \
             tc.tile_pool(name="p1ps", bufs=4, space="PSUM") as ps1:
            eng = 0
            for ci in range(0 if KPH == 2 else nch + 1):
                r0 = ci * RCH
                nr = RCH if ci < nch else rem
                if nr <= 0:
                    break
                nt = (nr + 127) // 128
                hc = pool1.tile([128, RCH], bf, tag="hc")
                nc.sync.dma_start(hc[:, 0:nr], hbT[:, r0:r0 + nr])
                kvc = pool1.tile([128, CH1, 2 * D], bf, tag="kvc")
                for t in range(nt):
                    nrt = min(128, nr - t * 128)
                    kps = ps1.tile([128, 2 * D], f32, tag="kps")
                    nc.tensor.matmul(kps[0:nrt, :], hc[:, t * 128:t * 128 + nrt],
                                     wkv_sb[:])
                    if eng == 0:
                        nc.vector.tensor_copy(kvc[0:nrt, t, :], kps[0:nrt, :])
                    else:
                        nc.scalar.copy(kvc[0:nrt, t, :], kps[0:nrt, :])
                    eng ^= 1
                if nr == RCH:
                    dstv = kv_table[r0:r0 + nr, :].rearrange(
                        "(t p) d -> p t d", p=128)
                    nc.scalar.dma_start(dstv, kvc[:])
                else:
                    for t in range(nt):
                        nrt = min(128, nr - t * 128)
                        nc.scalar.dma_start(
                            kv_table[r0 + t * 128:r0 + t * 128 + nrt, :],
                            kvc[0:nrt, t, :])

        # ---- phase 2: per-group fused attention + output block ----------
        kvpool = ctx.enter_context(tc.tile_pool(name="p2kv", bufs=5))
        prpool = ctx.enter_context(tc.tile_pool(name="p2pr", bufs=3))
        ipool = ctx.enter_context(tc.tile_pool(name="p2i", bufs=6))
        spool = ctx.enter_context(tc.tile_pool(name="p2s", bufs=4))
        psA = ctx.enter_context(tc.tile_pool(name="psA", bufs=2, space="PSUM"))
        psW = ctx.enter_context(tc.tile_pool(name="psW", bufs=2, space="PSUM"))
        psT = ctx.enter_context(tc.tile_pool(name="psT", bufs=1, space="PSUM"))
        psU = ctx.enter_context(tc.tile_pool(name="psU", bufs=1, space="PSUM"))
        psF = ctx.enter_context(tc.tile_pool(name="psF", bufs=1, space="PSUM"))
        psG = ctx.enter_context(tc.tile_pool(name="psG", bufs=1, space="PSUM"))

        def layernorm_apply(xin, g_off, skip, tag):
            """mean/var via bn_stats; apply on ACT (per-partition scale/bias).
            rstd = exp(-0.5*ln(var+eps)): keeps every ACT func in one act
            table set (no LoadActFuncSet churn)."""
            bn6 = spool.tile([128, 6], f32, tag=f"bn6{tag}")
            nc.vector.bn_stats(bn6[:], xin[:])
            ms = spool.tile([128, 2], f32, tag=f"ms{tag}")
            nc.vector.bn_aggr(ms[:], bn6[:])
            rstd = spool.tile([128, 1], f32, tag=f"rstd{tag}")
            nc.scalar.activation(rstd[:], ms[:, 1:2], AF.Abs_reciprocal_sqrt,
                                 bias=eps_sb[:, 0:1], scale=1.0)
            nmu = spool.tile([128, 1], f32, tag=f"nmu{tag}")
            nc.vector.tensor_scalar_mul(nmu[:], ms[:, 0:1], rstd[:, 0:1])
            nc.vector.tensor_scalar_mul(nmu[:], nmu[:], -1.0)
            y = spool.tile([128, D], f32, tag=f"y{tag}")
            nc.scalar.activation(y[:], xin[:], AF.Identity, bias=nmu[:, 0:1],
                                 scale=rstd[:, 0:1])
            if not skip:
                nc.vector.tensor_mul(y[:], y[:], ln_sb[:, g_off:g_off + D])
                nc.vector.tensor_add(y[:], y[:],
                                     ln_sb[:, g_off + D:g_off + 2 * D])
            return y

        def front(g):
            hTt = spool.tile([128, 128], bf, tag="hT")
            nc.sync.dma_start(hTt[:], hT_perm[:, g * GN:(g + 1) * GN])
            ht = spool.tile([128, D], f32, tag="h")
            nc.sync.dma_start(ht[:], h_perm[g * GN:(g + 1) * GN, :])
            qps = psA.tile([128, 128], f32, tag="psA")
            nc.tensor.matmul(qps[:], hTt[:], wq_sb[:])
            qsb = spool.tile([128, 128], bf, tag="q")
            nc.scalar.copy(qsb[:], qps[:])

            # per-subtile gather + scores + exp
            parts = []
            for (wa, wb), (bA, bB) in zip(SUBS[g], subbase[g]):
                wt = wa + wb
                cb = bA // 16
                idxt = ipool.tile([128, WCAP * 8], i16, tag="idx")
                nc.sync.dma_start(idxt[:, 0:wt * 8], kv_idx[:, cb:cb + wt * 8])
                kvt = kvpool.tile([128, WCAP, 2 * D], bf, tag="kvt")
                if wa:
                    nc.gpsimd.dma_gather(kvt[:, 0:wa, :], kv_table[0:AWIN, :],
                                         idxt[:, 0:wa * 8], wa * GN, wa * GN,
                                         2 * D, single_packet=False)
                if wb:
                    nc.gpsimd.dma_gather(kvt[:, wa:wt, :], kv_table[BBASE:ROWS, :],
                                         idxt[:, wa * 8:wt * 8], wb * GN,
                                         wb * GN, 2 * D, single_packet=False)

                # scores: e[p, w, h] = sum_jf q[p, h, jf] * k[p, w, h, jf]
                pr = prpool.tile([128, WCAP, 128], bf, tag="pr")
                nc.vector.tensor_mul(
                    pr[:, 0:wt, :].rearrange("p w (h j) -> p w h j", h=8, j=16),
                    kvt[:, 0:wt, 0:D].rearrange("p w (h j) -> p w h j", h=8, j=16),
                    qsb[:].rearrange("p (h j) -> p h j", h=8, j=16).unsqueeze(1)
                        .broadcast_to([128, wt, 8, 16]))
                # bf16 add-tree over jf (TensorTensor has the 2x DVE mode)
                pv = pr[:, 0:wt, :].rearrange("p w (h j) -> p w h j", h=8, j=16)
                nc.vector.tensor_add(pv[:, :, :, 0:8], pv[:, :, :, 0:8],
                                     pv[:, :, :, 8:16])
                nc.vector.tensor_add(pv[:, :, :, 0:4], pv[:, :, :, 0:4],
                                     pv[:, :, :, 4:8])
                nc.vector.tensor_add(pv[:, :, :, 0:2], pv[:, :, :, 0:2],
                                     pv[:, :, :, 2:4])
                nc.vector.tensor_add(pv[:, :, :, 0:1], pv[:, :, :, 0:1],
                                     pv[:, :, :, 1:2])
                exb = spool.tile([128, WCAP, 8], bf, tag="exb")
                nc.scalar.activation(exb[:, 0:wt, :].unsqueeze(3),
                                     pv[:, :, :, 0:1], AF.Exp, scale=0.25)
                parts.append((kvt, exb, wa, wb))
            return g, ht, parts

        def mid(st):
            g, ht, parts = st
            # denominator [node, h]; pad-corrected
            den = spool.tile([128, 8], f32, tag="den")
            nc.vector.reduce_sum(
                den[:], parts[0][1][:, 0:parts[0][2] + parts[0][3], :]
                .rearrange("p w h -> p h w"), axis=AX.X)
            for kvt, exb, wa, wb in parts[1:]:
                dpart = spool.tile([128, 8], f32, tag="dpart")
                nc.vector.reduce_sum(
                    dpart[:], exb[:, 0:wa + wb, :].rearrange("p w h -> p h w"),
                    axis=AX.X)
                nc.vector.tensor_add(den[:], den[:], dpart[:])
            den2 = spool.tile([128, 8], f32, tag="den2")
            nc.vector.tensor_scalar(den2[:], den[:], padc_sb[:, g:g + 1],
                                    None, AluOpType.subtract)
            nc.vector.tensor_scalar_max(den2[:], den2[:], 1e-30)
            rden = spool.tile([128, 8], f32, tag="rden")
            nc.vector.reciprocal(rden[:], den2[:])

            # alpha = exp/den folded into the exp tiles (O(w*8)), then
            # y = v * alpha and a^T accumulated on PE via identity matmuls
            numps = psA.tile([128, 128], f32, tag="psA")
            nsub = len(parts)
            for si, (kvt, exb, wa, wb) in enumerate(parts):
                wt = wa + wb
                exn = spool.tile([128, WCAP, 8], bf, tag="exn")
                nc.vector.tensor_mul(
                    exn[:, 0:wt, :], exb[:, 0:wt, :],
                    rden[:].unsqueeze(1).broadcast_to([128, wt, 8]))
                y = prpool.tile([128, WCAP, 128], bf, tag="y")
                nc.vector.tensor_mul(
                    y[:, 0:wt, :].rearrange("p w (j h) -> p w j h", j=16, h=8),
                    kvt[:, 0:wt, D:2 * D].rearrange("p w (j h) -> p w j h",
                                                    j=16, h=8),
                    exn[:, 0:wt, :].unsqueeze(2).broadcast_to([128, wt, 16, 8]))
                for j in range(wt):
                    nc.tensor.matmul(numps[:], y[:, j, :], id_sb[:],
                                     start=(si == 0 and j == 0),
                                     stop=(si == nsub - 1 and j == wt - 1))
            aT = spool.tile([128, 128], bf, tag="aT")
            nc.scalar.copy(aT[:], numps[:])

            wops = psW.tile([128, 128], f32, tag="psW")
            nc.tensor.matmul(wops[:], aT[:], wo_sb[:], start=True, stop=False)
            nc.tensor.matmul(wops[:], id32_sb[:], ht[:], start=False, stop=True)

            return g, wops

        def midB1(st):
            g, wops = st
            h1 = layernorm_apply(wops, 0, skip_ln1, "1")
            h1b = spool.tile([128, D], bf, tag="h1b")
            nc.scalar.copy(h1b[:], h1[:])
            h1ps = psT.tile([128, 128], bf, tag="psT")
            nc.tensor.transpose(h1ps[:], h1b[:], id_sb[:, :])
            h1T = spool.tile([128, 128], bf, tag="h1T")
            nc.vector.tensor_copy(h1T[:], h1ps[:])
            fps = psF.tile([128, 512], f32, tag="psF")
            for k in range(4):
                nc.tensor.matmul(fps[:, k * 128:(k + 1) * 128],
                                 w1_sb[:, k * 128:(k + 1) * 128], h1T[:],
                                 start=True, stop=True)
            rl = spool.tile([128, 512], bf, tag="rl")
            if add_b1:
                for k in range(4):
                    nc.scalar.activation(rl[:, k * 128:(k + 1) * 128],
                                         fps[:, k * 128:(k + 1) * 128],
                                         AF.Relu, bias=bff_sb[:, k, 0:1])
            else:
                nc.scalar.activation(rl[:], fps[:], AF.Relu)
            f2ps = psG.tile([128, 128], f32, tag="psG")
            for k in range(4):
                nc.tensor.matmul(f2ps[:], w2_sb[:, k, :],
                                 rl[:, k * 128:(k + 1) * 128],
                                 start=(k == 0), stop=(k == 3))
            f2b = spool.tile([128, 128], f32, tag="f2b")
            nc.scalar.copy(f2b[:], f2ps[:])
            return g, h1, f2b

        def midB2(st):
            g, h1, f2b = st
            fsl = psU.tile([128, 128], f32, tag="psU")
            nc.tensor.matmul(fsl[:], f2b[:], id32_sb[:], start=True, stop=False)
            nc.tensor.matmul(fsl[:], id32_sb[:], h1[:], start=False, stop=True)
            if add_b2:
                nc.vector.tensor_add(fsl[:], fsl[:], ln_sb[:, 4 * D:5 * D])
            h2 = layernorm_apply(fsl, 2 * D, skip_ln2, "2")
            nc.scalar.dma_start(out[g * GN:(g + 1) * GN, :], h2[:])

        if KPH != 1:
            halfg = (NG + 1) // 2
            sched = []
            for i in range(halfg):
                sched.append(i)
                if halfg + i < NG:
                    sched.append(halfg + i)
            from collections import deque
            fq, aq, bq = deque(), deque(), deque()
            fq.append(front(sched[0]))
            fq.append(front(sched[1]))
            aq.append(mid(fq.popleft()))
            fq.append(front(sched[2]))
            aq.append(mid(fq.popleft()))
            bq.append(midB1(aq.popleft()))
            for i in range(3, NG):
                fq.append(front(sched[i]))
                aq.append(mid(fq.popleft()))
                bq.append(midB1(aq.popleft()))
                midB2(bq.popleft())
            aq.append(mid(fq.popleft()))
            bq.append(midB1(aq.popleft()))
            midB2(bq.popleft())
            bq.append(midB1(aq.popleft()))
            midB2(bq.popleft())
            midB2(bq.popleft())

    # ---- per-core inputs -------------------------------------------------
    hbT_np = np.zeros((128, ROWS), bf16)
    hbT_np[:, 1:N + 1] = h.T.astype(bf16)
    wkv_np = np.concatenate([Wk, Wv], axis=1).astype(bf16)
    ln_rep2 = np.zeros((128, 5 * D), np.float32)
    ln_rep2[:, 0:D] = ln1_g
    ln_rep2[:, D:2 * D] = ln1_b
    ln_rep2[:, 2 * D:3 * D] = ln2_g
    ln_rep2[:, 3 * D:4 * D] = ln2_b
    ln_rep2[:, 4 * D:5 * D] = b2
    bias_ff_np = np.zeros((FF, 2), np.float32)
    bias_ff_np[:, 0] = b1
    id_np = np.eye(128, dtype=bf16)
    id32_np = np.eye(128, dtype=np.float32)

    in_maps = []
    perms = []
    for c in range(P):
        pr = preps[c]
        idx = _core_slots(pr, WA, WB, SUBS, subbase, TOT)
        padc_np = ((WA[:, None] - pr["dA_s"].reshape(NG, GN))
                   + (WB[:, None] - pr["dB_s"].reshape(NG, GN))
                   ).T.astype(np.float32)          # [GN, NG] -> [128, NG]
        hp = np.zeros((NSP, D), np.float32)
        hp[0:NS] = h[c * NS + pr["order"]]
        perms.append(pr["order"])
        in_maps.append({
            "hbT": hbT_np,
            "h_perm": hp,
            "hT_perm": np.ascontiguousarray(hp.T.astype(bf16)),
            "wkv_p": wkv_np,
            "wq_p": Wq.astype(bf16),
            "wo_p": Wo.astype(bf16),
            "w1_p": W1.astype(bf16),
            "w2_p": W2.astype(bf16),
            "ident": id_np,
            "ident32": id32_np,
            "ln_rep": ln_rep2,
            "bias_ff": bias_ff_np,
            "padc": padc_np,
            "kv_idx": _wrap16(idx),
        })

    nc.finalize()
    kernel.last_nc = nc
    res = run_bass_kernel_spmd(nc, in_maps, core_ids=list(range(P)),
                               trace=bool(int(os.environ.get("BASS_TRACE", "0"))))
    kernel.last_results = res
    full = np.empty((N, D), np.float32)
    for c in range(P):
        o = res.results[c]["out"]
        full[c * NS + perms[c]] = o[0:NS]
    return full


# revision 69
# speedup vs baseline: 3.6454x; 1.0172x over previous
import os
import sys
from contextlib import ExitStack

import numpy as np

sys.path.insert(0, "/opt/trn_rl_repo")

# Problem constants (hardcoded per harness contract)
N, E, D, H, DK, FF = 50000, 1600000, 128, 8, 16, 512
P = 8
NS = N // P            # 6250 nodes per core (dst shard)
GN = 128               # nodes per group (one SBUF partition each)
NG = (NS + GN - 1) // GN   # 49 groups
NSP = NG * GN          # 6272 padded node rows per core
EPS = 1e-5
WCAP = 40              # max gather blocks processed per subtile

# kv table rows: row 0 = zeros, rows 1..N = node r-1, row N+1 = zeros.
# window A = rows [0, 32768)        -> src s uses idx s+1      (s <= 32766)
# window B = rows [17234, 50002)    -> src s uses idx s-17233  (s >= 17233)
ROWS = N + 2
AWIN = 32768
BBASE = 17234
SLO = BBASE - 1        # s <  SLO: strictly window A
SHI = AWIN - 1         # s >= SHI: strictly window B
BPAD = N + 1 - BBASE   # 32767, idx of the zero row in window B

# v feature permutation: f' = jf*8 + h  (head index innermost, stride 1)
_FP = np.arange(D)
SRC_COL = (_FP % 8) * 16 + _FP // 8   # source feature for permuted position f'


def _wrap16(a):
    """int16 index array -> [128, L/16]: wrapped in 16 partitions, replicated
    across the 8 gpsimd cores (HW idx layout for dma_gather)."""
    a = np.asarray(a, np.int16)
    assert a.size % 16 == 0
    return np.ascontiguousarray(np.tile(a.reshape(-1, 16).T, (8, 1)))


def _rank_within(keys):
    """Rank of each element within its key group."""
    order = np.argsort(keys, kind="stable")
    ks = keys[order]
    first = np.r_[True, ks[1:] != ks[:-1]]
    idx_of_first = np.maximum.accumulate(np.where(first, np.arange(ks.size), 0))
    rank = np.arange(ks.size) - idx_of_first
    out = np.empty(keys.size, np.int64)
    out[order] = rank
    return out


def _split(n, k):
    q, r = divmod(n, k)
    return [q + (i < r) for i in range(k)]


def _prep_core(src, dst, c):
    """Per-core prep: balanced A/B window split + grouping by max(dA, dB)."""
    sel = (dst // NS) == c
    es = src[sel].astype(np.int64)
    ed = (dst[sel] - c * NS).astype(np.int64)

    isA = es < SLO
    isB = es >= SHI
    isF = ~(isA | isB)
    na = np.bincount(ed[isA], minlength=NS)
    nb = np.bincount(ed[isB], minlength=NS)
    nf = np.bincount(ed[isF], minlength=NS)
    d = na + nb + nf
    # flex edges sent to A, to balance dA vs dB
    x = np.clip((nb + nf - na + 1) // 2, 0, nf)
    frank = np.zeros(es.size, np.int64)
    frank[isF] = _rank_within(ed[isF])
    toA = isA | (isF & (frank < x[ed]))
    dA = np.bincount(ed[toA], minlength=NS)
    dB = d - dA

    # sort (desc, stable) by per-pass max degree (what actually sets W)
    order = np.argsort(-np.maximum(dA, dB), kind="stable")
    sortpos = np.empty(NS, np.int64)
    sortpos[order] = np.arange(NS)

    dA_s = np.zeros(NSP, np.int64)
    dB_s = np.zeros(NSP, np.int64)
    dA_s[sortpos] = dA
    dB_s[sortpos] = dB
    dAmax = dA_s.reshape(NG, GN).max(axis=1)
    dBmax = dB_s.reshape(NG, GN).max(axis=1)
    return dict(es=es, ed=ed, toA=toA, dA_s=dA_s, dB_s=dB_s,
                order=order, sortpos=sortpos, dAmax=dAmax, dBmax=dBmax)


def _make_subs(WA, WB):
    """Per-group subtile splits (wa_i, wb_i) with wa_i+wb_i <= WCAP, plus
    slot-stream bases."""
    SUBS, subbase = [], []
    off = 0
    for g in range(NG):
        n_sub = 1
        while True:
            was = _split(int(WA[g]), n_sub)
            wbs = _split(int(WB[g]), n_sub)
            if max(a + b for a, b in zip(was, wbs)) <= WCAP:
                break
            n_sub += 1
        SUBS.append(list(zip(was, wbs)))
        bl = []
        for wa, wb in SUBS[g]:
            bl.append((off, off + wa * GN))
            off += (wa + wb) * GN
        subbase.append(bl)
    return SUBS, subbase, off


def _core_slots(prep, WA, WB, SUBS, subbase, TOT):
    """Build the full int16 gather index stream for one core."""
    es, ed, toA = prep["es"], prep["ed"], prep["toA"]
    sortpos = prep["sortpos"]
    idx = np.empty(TOT, np.int16)
    # pad fills, and rank->slot-base LUTs per group
    lutA = np.zeros((NG, int(WA.max())), np.int64)
    lutB = np.zeros((NG, int(WB.max())), np.int64)
    for g in range(NG):
        ra = rb = 0
        for (wa, wb), (bA, bB) in zip(SUBS[g], subbase[g]):
            idx[bA:bA + wa * GN] = 0
            idx[bB:bB + wb * GN] = BPAD
            for j in range(wa):
                lutA[g, ra + j] = bA + j * GN
            for j in range(wb):
                lutB[g, rb + j] = bB + j * GN
            ra += wa
            rb += wb
    for sel, lut, val in (
        (toA, lutA, es + 1),
        (~toA, lutB, es - (BBASE - 1)),
    ):
        pos = sortpos[ed[sel]]
        g = pos // GN
        p = pos % GN
        r = _rank_within(pos)
        slot = lut[g, r] + p
        idx[slot] = val[sel].astype(np.int16)
    return idx


def kernel(**inputs):
    import concourse.bass as bass
    import concourse.tile as tile
    from concourse import bacc
    from concourse import mybir
    from concourse.bass_utils import run_bass_kernel_spmd
    from concourse.library_config import mlp as mlp_lib
    from concourse.alu_op_type import AluOpType
    import ml_dtypes

    bf16 = ml_dtypes.bfloat16
    f32 = mybir.dt.float32
    bf = mybir.dt.bfloat16
    i16 = mybir.dt.int16
    AF = mybir.ActivationFunctionType
    AX = mybir.AxisListType

    h = np.ascontiguousarray(np.asarray(inputs["h"], np.float32))
    src = np.asarray(inputs["src"], np.int32)
    dst = np.asarray(inputs["dst"], np.int32)
    Wq = np.asarray(inputs["Wq"], np.float32)
    Wk = np.asarray(inputs["Wk"], np.float32)
    Wv = np.asarray(inputs["Wv"], np.float32)[:, SRC_COL]
    Wo = np.asarray(inputs["Wo"], np.float32)[SRC_COL, :]
    ln1_g = np.asarray(inputs["ln1_g"], np.float32)
    ln1_b = np.asarray(inputs["ln1_b"], np.float32)
    ln2_g = np.asarray(inputs["ln2_g"], np.float32)
    ln2_b = np.asarray(inputs["ln2_b"], np.float32)
    W1 = np.asarray(inputs["W1"], np.float32)
    b1 = np.asarray(inputs["b1"], np.float32)
    W2 = np.asarray(inputs["W2"], np.float32)
    b2 = np.asarray(inputs["b2"], np.float32)

    # ---- host-side sharding / index prep --------------------------------
    preps = [_prep_core(src, dst, c) for c in range(P)]
    WA = np.maximum(np.max([p["dAmax"] for p in preps], axis=0), 1)
    WB = np.maximum(np.max([p["dBmax"] for p in preps], axis=0), 1)
    SUBS, subbase, TOT = _make_subs(WA, WB)
    TOTC = TOT // 16

    skip_ln1 = not (np.any(ln1_g != 1.0) or np.any(ln1_b != 0.0))
    skip_ln2 = not (np.any(ln2_g != 1.0) or np.any(ln2_b != 0.0))
    add_b1 = bool(np.any(b1 != 0.0))
    add_b2 = bool(np.any(b2 != 0.0))

    # ---- device program --------------------------------------------------
    nc = bacc.Bacc(None)
    hbT = nc.declare_dram_parameter("hbT", [128, ROWS], bf, isOutput=False)
    h_perm = nc.declare_dram_parameter("h_perm", [NSP, D], f32, isOutput=False)
    hT_perm = nc.declare_dram_parameter("hT_perm", [128, NSP], bf, isOutput=False)
    wkv_p = nc.declare_dram_parameter("wkv_p", [D, 2 * D], bf, isOutput=False)
    wq_p = nc.declare_dram_parameter("wq_p", [D, D], bf, isOutput=False)
    wo_p = nc.declare_dram_parameter("wo_p", [D, D], bf, isOutput=False)
    w1_p = nc.declare_dram_parameter("w1_p", [D, FF], bf, isOutput=False)
    w2_p = nc.declare_dram_parameter("w2_p", [FF, D], bf, isOutput=False)
    ident = nc.declare_dram_parameter("ident", [128, 128], bf, isOutput=False)
    ident32 = nc.declare_dram_parameter("ident32", [128, 128], f32, isOutput=False)
    ln_rep = nc.declare_dram_parameter("ln_rep", [128, 5 * D], f32, isOutput=False)
    bias_ff = nc.declare_dram_parameter("bias_ff", [FF, 2], f32, isOutput=False)
    padc = nc.declare_dram_parameter("padc", [128, NG], f32, isOutput=False)
    kv_idx = nc.declare_dram_parameter("kv_idx", [128, TOTC], i16, isOutput=False)
    out = nc.declare_dram_parameter("out", [NSP, D], f32, isOutput=True)

    kv_table = nc.dram_tensor("kv_table", [ROWS, 2 * D], bf)

    with tile.TileContext(nc) as tc, ExitStack() as ctx:
        nc.gpsimd.load_library(mlp_lib)
        cpool = ctx.enter_context(tc.tile_pool(name="consts", bufs=1))
        wkv_sb = cpool.tile([128, 2 * D], bf)
        nc.sync.dma_start(wkv_sb[:], wkv_p[:, :])
        wq_sb = cpool.tile([128, D], bf)
        nc.sync.dma_start(wq_sb[:], wq_p[:, :])
        wo_sb = cpool.tile([128, D], bf)
        nc.sync.dma_start(wo_sb[:], wo_p[:, :])
        w1_sb = cpool.tile([128, FF], bf)
        nc.sync.dma_start(w1_sb[:], w1_p[:, :])
        w2_sb = cpool.tile([128, 4, D], bf)
        for k in range(4):
            nc.sync.dma_start(w2_sb[:, k, :], w2_p[k * 128:(k + 1) * 128, :])
        id_sb = cpool.tile([128, 128], bf)
        nc.sync.dma_start(id_sb[:], ident[:, :])
        id32_sb = cpool.tile([128, 128], f32)
        nc.sync.dma_start(id32_sb[:], ident32[:, :])
        ln_sb = cpool.tile([128, 5 * D], f32)
        nc.sync.dma_start(ln_sb[:], ln_rep[:, :])
        bff_sb = cpool.tile([128, 4, 2], f32)
        for k in range(4):
            nc.sync.dma_start(bff_sb[:, k, :], bias_ff[k * 128:(k + 1) * 128, :])
        padc_sb = cpool.tile([128, NG], f32)
        nc.sync.dma_start(padc_sb[:], padc[:, :])
        eps_sb = cpool.tile([128, 1], f32)
        nc.vector.memset(eps_sb[:], EPS)

        # ---- phase 1: build kv table [k_nat | v_perm] for all table rows -
        KPH = int(os.environ.get("KPH", "0"))  # 1: phase1 only, 2: phase2 only
        CH1 = 16                      # 128-row tiles per chunk
        RCH = CH1 * 128              # 1024 rows per chunk
        nch = ROWS // RCH
        rem = ROWS - nch * RCH
        with tc.tile_pool(name="p1", bufs=3) as pool1, \
             tc.tile_pool(name="p1ps", bufs=8, space="PSUM") as ps1:
            eng = 0
            for ci in range(0 if KPH == 2 else nch + 1):
                r0 = ci * RCH
                nr = RCH if ci < nch else rem
                if nr <= 0:
                    break
                nt = (nr + 127) // 128
                hc = pool1.tile([128, RCH], bf, tag="hc")
                nc.sync.dma_start(hc[:, 0:nr], hbT[:, r0:r0 + nr])
                kvc = pool1.tile([128, CH1, 2 * D], bf, tag="kvc")
                for t in range(nt):
                    nrt = min(128, nr - t * 128)
                    kps = ps1.tile([128, 2 * D], f32, tag="kps")
                    nc.tensor.matmul(kps[0:nrt, :], hc[:, t * 128:t * 128 + nrt],
                                     wkv_sb[:])
                    if eng == 0:
                        nc.vector.tensor_copy(kvc[0:nrt, t, :], kps[0:nrt, :])
                    else:
                        nc.scalar.copy(kvc[0:nrt, t, :], kps[0:nrt, :])
                    eng ^= 1
                if nr == RCH:
                    dstv = kv_table[r0:r0 + nr, :].rearrange(
                        "(t p) d -> p t d", p=128)
                    if ci % 2 == 0:
                        nc.scalar.dma_start(dstv, kvc[:])
                    else:
                        nc.sync.dma_start(dstv, kvc[:])
                else:
                    for t in range(nt):
                        nrt = min(128, nr - t * 128)
                        nc.scalar.dma_start(
                            kv_table[r0 + t * 128:r0 + t * 128 + nrt, :],
                            kvc[0:nrt, t, :])

        # ---- phase 2: per-group fused attention + output block ----------
        kvpool = ctx.enter_context(tc.tile_pool(name="p2kv", bufs=5))
        prpool = ctx.enter_context(tc.tile_pool(name="p2pr", bufs=3))
        ipool = ctx.enter_context(tc.tile_pool(name="p2i", bufs=6))
        spool = ctx.enter_context(tc.tile_pool(name="p2s", bufs=4))
        psA = ctx.enter_context(tc.tile_pool(name="psA", bufs=2, space="PSUM"))
        psW = ctx.enter_context(tc.tile_pool(name="psW", bufs=2, space="PSUM"))
        psT = ctx.enter_context(tc.tile_pool(name="psT", bufs=1, space="PSUM"))
        psU = ctx.enter_context(tc.tile_pool(name="psU", bufs=1, space="PSUM"))
        psF = ctx.enter_context(tc.tile_pool(name="psF", bufs=1, space="PSUM"))
        psG = ctx.enter_context(tc.tile_pool(name="psG", bufs=1, space="PSUM"))

        def layernorm_apply(xin, g_off, skip, tag):
            """mean/var via bn_stats; apply on ACT (per-partition scale/bias).
            rstd = exp(-0.5*ln(var+eps)): keeps every ACT func in one act
            table set (no LoadActFuncSet churn)."""
            bn6 = spool.tile([128, 6], f32, tag=f"bn6{tag}")
            nc.vector.bn_stats(bn6[:], xin[:])
            ms = spool.tile([128, 2], f32, tag=f"ms{tag}")
            nc.vector.bn_aggr(ms[:], bn6[:])
            rstd = spool.tile([128, 1], f32, tag=f"rstd{tag}")
            nc.scalar.activation(rstd[:], ms[:, 1:2], AF.Abs_reciprocal_sqrt,
                                 bias=eps_sb[:, 0:1], scale=1.0)
            nmu = spool.tile([128, 1], f32, tag=f"nmu{tag}")
            nc.vector.tensor_scalar_mul(nmu[:], ms[:, 0:1], rstd[:, 0:1])
            nc.vector.tensor_scalar_mul(nmu[:], nmu[:], -1.0)
            y = spool.tile([128, D], f32, tag=f"y{tag}")
            nc.scalar.activation(y[:], xin[:], AF.Identity, bias=nmu[:, 0:1],
                                 scale=rstd[:, 0:1])
            if not skip:
                nc.vector.tensor_mul(y[:], y[:], ln_sb[:, g_off:g_off + D])
                nc.vector.tensor_add(y[:], y[:],
                                     ln_sb[:, g_off + D:g_off + 2 * D])
            return y

        def front(g):
            hTt = spool.tile([128, 128], bf, tag="hT")
            nc.sync.dma_start(hTt[:], hT_perm[:, g * GN:(g + 1) * GN])
            ht = spool.tile([128, D], f32, tag="h")
            nc.sync.dma_start(ht[:], h_perm[g * GN:(g + 1) * GN, :])
            qps = psA.tile([128, 128], f32, tag="psA")
            nc.tensor.matmul(qps[:], hTt[:], wq_sb[:])
            qsb = spool.tile([128, 128], bf, tag="q")
            nc.scalar.copy(qsb[:], qps[:])

            # per-subtile gather + scores + exp
            parts = []
            for (wa, wb), (bA, bB) in zip(SUBS[g], subbase[g]):
                wt = wa + wb
                cb = bA // 16
                idxt = ipool.tile([128, WCAP * 8], i16, tag="idx")
                nc.sync.dma_start(idxt[:, 0:wt * 8], kv_idx[:, cb:cb + wt * 8])
                kvt = kvpool.tile([128, WCAP, 2 * D], bf, tag="kvt")
                if wa:
                    nc.gpsimd.dma_gather(kvt[:, 0:wa, :], kv_table[0:AWIN, :],
                                         idxt[:, 0:wa * 8], wa * GN, wa * GN,
                                         2 * D, single_packet=False)
                if wb:
                    nc.gpsimd.dma_gather(kvt[:, wa:wt, :], kv_table[BBASE:ROWS, :],
                                         idxt[:, wa * 8:wt * 8], wb * GN,
                                         wb * GN, 2 * D, single_packet=False)

                # scores: e[p, w, h] = sum_jf q[p, h, jf] * k[p, w, h, jf]
                pr = prpool.tile([128, WCAP, 128], bf, tag="pr")
                nc.vector.tensor_mul(
                    pr[:, 0:wt, :].rearrange("p w (h j) -> p w h j", h=8, j=16),
                    kvt[:, 0:wt, 0:D].rearrange("p w (h j) -> p w h j", h=8, j=16),
                    qsb[:].rearrange("p (h j) -> p h j", h=8, j=16).unsqueeze(1)
                        .broadcast_to([128, wt, 8, 16]))
                # bf16 add-tree over jf (TensorTensor has the 2x DVE mode)
                pv = pr[:, 0:wt, :].rearrange("p w (h j) -> p w h j", h=8, j=16)
                nc.vector.tensor_add(pv[:, :, :, 0:8], pv[:, :, :, 0:8],
                                     pv[:, :, :, 8:16])
                nc.vector.tensor_add(pv[:, :, :, 0:4], pv[:, :, :, 0:4],
                                     pv[:, :, :, 4:8])
                nc.vector.tensor_add(pv[:, :, :, 0:2], pv[:, :, :, 0:2],
                                     pv[:, :, :, 2:4])
                nc.vector.tensor_add(pv[:, :, :, 0:1], pv[:, :, :, 0:1],
                                     pv[:, :, :, 1:2])
                exb = spool.tile([128, WCAP, 8], bf, tag="exb")
                nc.scalar.activation(exb[:, 0:wt, :].unsqueeze(3),
                                     pv[:, :, :, 0:1], AF.Exp, scale=0.25)
                parts.append((kvt, exb, wa, wb))
            return g, ht, parts

        def mid(st):
            g, ht, parts = st
            # denominator [node, h]; pad-corrected
            den = spool.tile([128, 8], f32, tag="den")
            nc.vector.reduce_sum(
                den[:], parts[0][1][:, 0:parts[0][2] + parts[0][3], :]
                .rearrange("p w h -> p h w"), axis=AX.X)
            for kvt, exb, wa, wb in parts[1:]:
                dpart = spool.tile([128, 8], f32, tag="dpart")
                nc.vector.reduce_sum(
                    dpart[:], exb[:, 0:wa + wb, :].rearrange("p w h -> p h w"),
                    axis=AX.X)
                nc.vector.tensor_add(den[:], den[:], dpart[:])
            den2 = spool.tile([128, 8], f32, tag="den2")
            nc.vector.tensor_scalar(den2[:], den[:], padc_sb[:, g:g + 1],
                                    None, AluOpType.subtract)
            nc.vector.tensor_scalar_max(den2[:], den2[:], 1e-30)
            rden = spool.tile([128, 8], f32, tag="rden")
            nc.vector.reciprocal(rden[:], den2[:])

            # alpha = exp/den folded into the exp tiles (O(w*8)), then
            # y = v * alpha and a^T accumulated on PE via identity matmuls
            numps = psA.tile([128, 128], f32, tag="psA")
            nsub = len(parts)
            for si, (kvt, exb, wa, wb) in enumerate(parts):
                wt = wa + wb
                exn = spool.tile([128, WCAP, 8], bf, tag="exn")
                nc.vector.tensor_mul(
                    exn[:, 0:wt, :], exb[:, 0:wt, :],
                    rden[:].unsqueeze(1).broadcast_to([128, wt, 8]))
                y = prpool.tile([128, WCAP, 128], bf, tag="y")
                nc.vector.tensor_mul(
                    y[:, 0:wt, :].rearrange("p w (j h) -> p w j h", j=16, h=8),
                    kvt[:, 0:wt, D:2 * D].rearrange("p w (j h) -> p w j h",
                                                    j=16, h=8),
                    exn[:, 0:wt, :].unsqueeze(2).broadcast_to([128, wt, 16, 8]))
                for j in range(wt):
                    nc.tensor.matmul(numps[:], y[:, j, :], id_sb[:],
                                     start=(si == 0 and j == 0),
                                     stop=(si == nsub - 1 and j == wt - 1))
            aT = spool.tile([128, 128], bf, tag="aT")
            nc.scalar.copy(aT[:], numps[:])

            wops = psW.tile([128, 128], f32, tag="psW")
            nc.tensor.matmul(wops[:], aT[:], wo_sb[:], start=True, stop=False)
            nc.tensor.matmul(wops[:], id32_sb[:], ht[:], start=False, stop=True)

            return g, wops

        def midB1(st):
            g, wops = st
            h1 = layernorm_apply(wops, 0, skip_ln1, "1")
            h1b = spool.tile([128, D], bf, tag="h1b")
            nc.scalar.copy(h1b[:], h1[:])
            h1ps = psT.tile([128, 128], bf, tag="psT")
            nc.tensor.transpose(h1ps[:], h1b[:], id_sb[:, :])
            h1T = spool.tile([128, 128], bf, tag="h1T")
            nc.vector.tensor_copy(h1T[:], h1ps[:])
            fps = psF.tile([128, 512], f32, tag="psF")
            for k in range(4):
                nc.tensor.matmul(fps[:, k * 128:(k + 1) * 128],
                                 w1_sb[:, k * 128:(k + 1) * 128], h1T[:],
                                 start=True, stop=True)
            rl = spool.tile([128, 512], bf, tag="rl")
            if add_b1:
                for k in range(4):
                    nc.scalar.activation(rl[:, k * 128:(k + 1) * 128],
                                         fps[:, k * 128:(k + 1) * 128],
                                         AF.Relu, bias=bff_sb[:, k, 0:1])
            else:
                nc.scalar.activation(rl[:], fps[:], AF.Relu)
            f2ps = psG.tile([128, 128], f32, tag="psG")
            for k in range(4):
                nc.tensor.matmul(f2ps[:], w2_sb[:, k, :],
                                 rl[:, k * 128:(k + 1) * 128],
                                 start=(k == 0), stop=(k == 3))
            f2b = spool.tile([128, 128], f32, tag="f2b")
            nc.scalar.copy(f2b[:], f2ps[:])
            return g, h1, f2b

        def midB2(st):
            g, h1, f2b = st
            fsl = psU.tile([128, 128], f32, tag="psU")
            nc.tensor.matmul(fsl[:], f2b[:], id32_sb[:], start=True, stop=False)
            nc.tensor.matmul(fsl[:], id32_sb[:], h1[:], start=False, stop=True)
            if add_b2:
                nc.vector.tensor_add(fsl[:], fsl[:], ln_sb[:, 4 * D:5 * D])
            h2 = layernorm_apply(fsl, 2 * D, skip_ln2, "2")
            nc.scalar.dma_start(out[g * GN:(g + 1) * GN, :], h2[:])

        if KPH != 1:
            halfg = (NG + 1) // 2
            sched = []
            for i in range(halfg):
                sched.append(i)
                if halfg + i < NG:
                    sched.append(halfg + i)
            from collections import deque
            fq, aq, bq = deque(), deque(), deque()
            fq.append(front(sched[0]))
            fq.append(front(sched[1]))
            aq.append(mid(fq.popleft()))
            fq.append(front(sched[2]))
            aq.append(mid(fq.popleft()))
            bq.append(midB1(aq.popleft()))
            for i in range(3, NG):
                fq.append(front(sched[i]))
                aq.append(mid(fq.popleft()))
                bq.append(midB1(aq.popleft()))
                midB2(bq.popleft())
            aq.append(mid(fq.popleft()))
            bq.append(midB1(aq.popleft()))
            midB2(bq.popleft())
            bq.append(midB1(aq.popleft()))
            midB2(bq.popleft())
            midB2(bq.popleft())

    # ---- per-core inputs -------------------------------------------------
    hbT_np = np.zeros((128, ROWS), bf16)
    hbT_np[:, 1:N + 1] = h.T.astype(bf16)
    wkv_np = np.concatenate([Wk, Wv], axis=1).astype(bf16)
    ln_rep2 = np.zeros((128, 5 * D), np.float32)
    ln_rep2[:, 0:D] = ln1_g
    ln_rep2[:, D:2 * D] = ln1_b
    ln_rep2[:, 2 * D:3 * D] = ln2_g
    ln_rep2[:, 3 * D:4 * D] = ln2_b
    ln_rep2[:, 4 * D:5 * D] = b2
    bias_ff_np = np.zeros((FF, 2), np.float32)
    bias_ff_np[:, 0] = b1
    id_np = np.eye(128, dtype=bf16)
    id32_np = np.eye(128, dtype=np.float32)

    in_maps = []
    perms = []
    for c in range(P):
        pr = preps[c]
        idx = _core_slots(pr, WA, WB, SUBS, subbase, TOT)
        padc_np = ((WA[:, None] - pr["dA_s"].reshape(NG, GN))
                   + (WB[:, None] - pr["dB_s"].reshape(NG, GN))
                   ).T.astype(np.float32)          # [GN, NG] -> [128, NG]
        hp = np.zeros((NSP, D), np.float32)
        hp[0:NS] = h[c * NS + pr["order"]]
        perms.append(pr["order"])
        in_maps.append({
            "hbT": hbT_np,
            "h_perm": hp,
            "hT_perm": np.ascontiguousarray(hp.T.astype(bf16)),
            "wkv_p": wkv_np,
            "wq_p": Wq.astype(bf16),
            "wo_p": Wo.astype(bf16),
            "w1_p": W1.astype(bf16),
            "w2_p": W2.astype(bf16),
            "ident": id_np,
            "ident32": id32_np,
            "ln_rep": ln_rep2,
            "bias_ff": bias_ff_np,
            "padc": padc_np,
            "kv_idx": _wrap16(idx),
        })

    nc.finalize()
    kernel.last_nc = nc
    res = run_bass_kernel_spmd(nc, in_maps, core_ids=list(range(P)),
                               trace=bool(int(os.environ.get("BASS_TRACE", "0"))))
    kernel.last_results = res
    full = np.empty((N, D), np.float32)
    for c in range(P):
        o = res.results[c]["out"]
        full[c * NS + perms[c]] = o[0:NS]
    return full
